# revision 1
# baseline (speedup 1.0000x reference)
"""Trainium2 Bass kernel for nn_BAFM_BRB_65249143161427 (segment_reduce).

Reference semantics: downsample x0/x1 by 8 (nearest), compute directional
running segment means between zero-boundaries of x1 along rows and columns,
sum the 4 directional terms, upsample by 8.

Sharding: pure data parallel — image n -> core n (N=8, 8 cores).
Each core processes one full 2048x2048 image.

Structure (per core): the 16 MB output store stream dominates (~47 us at
360 GB/s), so the program is ordered to start storing as early as possible:
load -> h-pass(tile0) -> transposes -> v-pass(col0) -> store block (0,0),
with the remaining passes' DVE work hidden under the store stream.
"""
import sys

sys.path.insert(0, "/opt/trn_rl_repo")

import numpy as np

H = W = 2048
S = 8
HD, WD = H // S, W // S      # 256 x 256 downsampled grid
P = 128                      # SBUF partitions
NT = HD // P                 # 2 row-tiles of the downsampled grid
N_CORES = 8

_CACHE = {}


def _revap(ap2d):
    """Reverse the last (free) dim of a 2D access pattern."""
    from concourse.ap import AP

    pairs = [list(p) for p in ap2d.ap]
    step, n = pairs[-1]
    return AP(ap2d.tensor, ap2d.offset + (n - 1) * step, pairs[:-1] + [[-step, n]])


def _bcast8(ap2d):
    """Append a step-0 count-8 inner dim (each element replicated 8x)."""
    from concourse.ap import AP

    pairs = [list(p) for p in ap2d.ap]
    return AP(ap2d.tensor, ap2d.offset, pairs + [[0, 8]])


def build_program(loop_n: int = 1, fast_recip=False, pool_elem=True):
    import concourse.bacc as bacc
    import concourse.tile as tile
    from concourse import mybir
    from concourse.masks import make_identity
    from contextlib import ExitStack

    f32 = mybir.dt.float32
    i32 = mybir.dt.int32
    MUL = mybir.AluOpType.mult
    ADD = mybir.AluOpType.add
    NE = mybir.AluOpType.not_equal

    # Bacc (not raw Bass): its compile() splits multi-wait sync commands,
    # which TRN2 engines (1 wait/instruction) require.
    nc = bacc.Bacc("TRN2")
    x0 = nc.declare_dram_parameter("x0", [H, W], f32, isOutput=False)
    x1 = nc.declare_dram_parameter("x1", [H, W], i32, isOutput=False)
    y = nc.declare_dram_parameter("y", [H, W], f32, isOutput=True)

    # DRAM viewed with rows grouped by 8: [256, 8, 2048]
    x0g = x0[:].rearrange("(r e) w -> r e w", e=8)
    x1g = x1[:].rearrange("(r e) w -> r e w", e=8)
    yg = y[:].rearrange("(r e) w -> r e w", e=8)

    with tile.TileContext(nc) as tc:
        with ExitStack() as ctx:
            const_pool = ctx.enter_context(tc.tile_pool(name="const", bufs=1))
            io_pool = ctx.enter_context(tc.tile_pool(name="io", bufs=1))
            work = ctx.enter_context(tc.tile_pool(name="work", bufs=1))
            # PSUM is fully subscribed: this pool (2 tags x 2 bufs = 4
            # banks) + psum_x (2 tags x 2 bufs = 4 banks) = all 8 banks
            psum = ctx.enter_context(
                tc.tile_pool(name="psum", bufs=2, space="PSUM")
            )
            psum_x = ctx.enter_context(
                tc.tile_pool(name="psum_x", bufs=2, space="PSUM")
            )

            ident_g = const_pool.tile([P, P], f32)
            make_identity(nc, ident_g[:])
            ident = const_pool.tile([P, P], f32)
            nc.vector.tensor_copy(ident[:], ident_g[:])
            ones = const_pool.tile([P, WD], f32)
            nc.gpsimd.memset(ones[:], 1.0)

            elem_eng = nc.gpsimd if pool_elem else nc.vector

            def body():
                # ---- tiles ----
                am = [work.tile([P, WD], f32, tag=f"am{t}", name=f"am{t}") for t in range(NT)]
                # transposed x stays in PSUM (scans read it directly);
                # only the transposed mask is staged to SBUF
                xT = [psum_x.tile([P, HD], f32, tag=f"xT{c}", name=f"xT{c}") for c in range(NT)]
                aT = [work.tile([P, HD], f32, tag=f"aT{c}", name=f"aT{c}") for c in range(NT)]
                yh = [None] * NT    # horizontal-pass results
                yv = [None] * NT    # vertical-pass results (transposed)
                ye = [io_pool.tile([P, W], f32, tag=f"ye{t}", name=f"ye{t}") for t in range(NT)]

                def load_x1(t):
                    x1r = io_pool.tile([P, W], i32, tag=f"x1r{t}")
                    nc.sync.dma_start(
                        out=x1r[:], in_=x1g[t * P:(t + 1) * P, 0, :]
                    )
                    # (DVE: Pool rejects TensorScalarPtr in codegen)
                    nc.vector.tensor_scalar(
                        out=am[t][:],
                        in0=x1r[:].rearrange("p (a b) -> p a b", b=8)[:, :, 0],
                        scalar1=0, scalar2=None, op0=NE,
                    )

                x0r = [None] * NT

                def load_x0_half(t, h):
                    # half-column DMA; consumers read the row tile directly
                    # through stride-8 views (no downsample copy, no extra
                    # DMA-sem hop on the critical path)
                    if x0r[t] is None:
                        x0r[t] = io_pool.tile(
                            [P, W], f32, tag=f"x0r{t}", name=f"x0r{t}")
                    hw_ = W // 2
                    nc.sync.dma_start(
                        out=x0r[t][:, h * hw_:(h + 1) * hw_],
                        in_=x0g[t * P:(t + 1) * P, 0, h * hw_:(h + 1) * hw_],
                    )

                def xdv(t, lo=0, hi=WD):
                    """Stride-8 view of x0r[t] covering downsampled cols
                    [lo, hi)."""
                    return x0r[t][:, lo * 8:hi * 8].rearrange(
                        "p (a b) -> p a b", b=8)[:, :, 0]

                def seg_counts(at, pf):
                    """Mask-only stage: count reciprocals + invalid mask q."""
                    c_lr = work.tile([P, WD], f32, tag=f"clr{pf}")
                    nb = work.tile([P, WD], f32, tag=f"nb{pf}")
                    c_rl = work.tile([P, WD], f32, tag=f"crl{pf}")  # reversed
                    na = work.tile([P, WD], f32, tag=f"na{pf}")     # reversed
                    a_r = _revap(at[:])
                    ttscan = nc.vector.tensor_tensor_scan
                    ttscan(c_lr[:], at[:], ones[:], 0.0, MUL, ADD)
                    ttscan(c_rl[:], a_r, ones[:], 0.0, MUL, ADD)
                    ttscan(nb[:], at[:], at[:], 1.0, MUL, MUL)
                    ttscan(na[:], a_r, a_r, 1.0, MUL, MUL)
                    ilr = work.tile([P, WD], f32, tag=f"ilr{pf}")
                    irl = work.tile([P, WD], f32, tag=f"irl{pf}")
                    if fast_recip:
                        nc.vector.reciprocal_approx_fast(ilr[:], c_lr[:])
                        nc.vector.reciprocal_approx_fast(irl[:], c_rl[:])
                    else:
                        scr = work.tile([P, WD], f32, tag=f"scr{pf}")
                        nc.vector.reciprocal_approx_accurate(
                            ilr[:], c_lr[:], scr[:])
                        nc.vector.reciprocal_approx_accurate(
                            irl[:], c_rl[:], scr[:])
                    # q = nb + reverse(na): nonzero -> no enclosing boundary
                    # (int32: BIR requires an integer predicate dtype)
                    # (DVE: Pool rejects f32->i32 dtype-converting TensorTensor)
                    q = work.tile([P, WD], i32, tag=f"q{pf}")
                    nc.vector.tensor_tensor(q[:], nb[:], _revap(na[:]), ADD)
                    return ilr, irl, q

                def seg_sums(xt, at, cnts, pf):
                    """Value stage: segment sums -> means -> m with fallback."""
                    ilr, irl, q = cnts
                    s_lr = work.tile([P, WD], f32, tag=f"slr{pf}")
                    s_rl = work.tile([P, WD], f32, tag=f"srl{pf}")  # reversed
                    a_r = _revap(at[:])
                    x_r = _revap(xt[:])
                    ttscan = nc.vector.tensor_tensor_scan
                    ttscan(s_lr[:], at[:], xt[:], 0.0, MUL, ADD)
                    ttscan(s_rl[:], a_r, x_r, 0.0, MUL, ADD)
                    elem_eng.tensor_tensor(s_lr[:], s_lr[:], ilr[:], MUL)
                    elem_eng.tensor_tensor(s_rl[:], s_rl[:], irl[:], MUL)
                    m = work.tile([P, WD], f32, tag=f"m{pf}")
                    nc.vector.tensor_tensor(m[:], s_lr[:], _revap(s_rl[:]), ADD)
                    two_x = work.tile([P, WD], f32, tag=f"tx{pf}")
                    nc.scalar.mul(two_x[:], xt[:], 2.0)
                    nc.vector.copy_predicated(m[:], q[:], two_x[:])
                    return m

                def seg_sums_split(xt, at, cnts, pf, tail_eng=None):
                    """Critical-path variant: tail split into column halves —
                    half 0 (needed by the first store block) first; second
                    half's arithmetic goes to Pool to keep DVE clear.
                    Returns (m, finish_fn)."""
                    ilr, irl, q = cnts
                    s_lr = work.tile([P, WD], f32, tag=f"slr{pf}")
                    s_rl = work.tile([P, WD], f32, tag=f"srl{pf}")  # reversed
                    ttscan = nc.vector.tensor_tensor_scan
                    ttscan(s_lr[:], at[:], xt[:], 0.0, MUL, ADD)
                    ttscan(s_rl[:], _revap(at[:]), _revap(xt[:]), 0.0, MUL, ADD)
                    m = work.tile([P, WD], f32, tag=f"m{pf}")
                    two_x = work.tile([P, WD], f32, tag=f"tx{pf}")
                    nc.scalar.mul(two_x[:], xt[:], 2.0)
                    hd_ = WD // 2

                    def half(h, eng):
                        lo, hi = h * hd_, (h + 1) * hd_
                        rlo, rhi = WD - hi, WD - lo  # mirrored slice (rev space)
                        eng.tensor_tensor(
                            s_lr[:, lo:hi], s_lr[:, lo:hi], ilr[:, lo:hi], MUL)
                        eng.tensor_tensor(
                            s_rl[:, rlo:rhi], s_rl[:, rlo:rhi],
                            irl[:, rlo:rhi], MUL)
                        eng.tensor_tensor(
                            m[:, lo:hi], s_lr[:, lo:hi],
                            _revap(s_rl[:, rlo:rhi]), ADD)
                        nc.vector.copy_predicated(
                            m[:, lo:hi], q[:, lo:hi], two_x[:, lo:hi])

                    half(0, tail_eng or nc.vector)
                    return m, (lambda: half(1, nc.vector))

                def transpose_a(c):
                    """Transposed mask -> aT[c] (SBUF, bounced via PSUM:
                    scan data0 and data1 cannot both live in PSUM)."""
                    for t in range(NT):
                        pb = psum.tile([P, P], f32, tag="ptr")
                        nc.tensor.transpose(
                            pb[:], am[t][:, c * P:(c + 1) * P], ident[:]
                        )
                        nc.scalar.copy(aT[c][:, t * P:(t + 1) * P], pb[:])

                def transpose_x(c):
                    """Transpose x straight into the PSUM tile the vertical
                    sum-scans read."""
                    for t in range(NT):
                        nc.tensor.transpose(
                            xT[c][:, t * P:(t + 1) * P],
                            xdv(t, c * P, (c + 1) * P), ident[:],
                        )

                def combine_store(t, c, first=False):
                    """y block (t,c) = yh[t][:,cP:] + yv[c][:,tP:]^T;
                    expand 8x8, store 8 row-replicas. first=True keeps the
                    expansion on DVE (skips the ACT hop on the path that
                    opens the store stream)."""
                    pb = psum.tile([P, P], f32, tag="ptb")
                    nc.tensor.transpose(
                        pb[:], yv[c][:, t * P:(t + 1) * P], ident[:]
                    )
                    cw = W // NT
                    ye_view = ye[t][:, c * cw:(c + 1) * cw].rearrange(
                        "p (a b) -> p a b", b=8)
                    ysum = work.tile([P, P], f32, tag=f"ys{t}{c}")
                    nc.vector.tensor_tensor(
                        ysum[:], yh[t][:, c * P:(c + 1) * P], pb[:], ADD
                    )
                    if first:
                        nc.vector.tensor_copy(ye_view, _bcast8(ysum[:]))
                    else:
                        nc.scalar.copy(ye_view, _bcast8(ysum[:]))
                    # 8 parallel row-replica DMAs: on real HW multiple
                    # in-flight DMAs fan out across queues (a single
                    # broadcast-source DMA measured ~20us slower)
                    # alternate HWDGE issuers (SP / ACT) so descriptor
                    # generation for the store stream runs on two sequencers
                    for k in range(8):
                        issuer = nc.sync if k % 2 == 0 else nc.scalar
                        issuer.dma_start(
                            out=yg[t * P:(t + 1) * P, k, c * cw:(c + 1) * cw],
                            in_=ye[t][:, c * cw:(c + 1) * cw],
                        )

                # ---- ordered for earliest store start ----
                # masks first (x1 loads lead), count stages run during x0
                # loads, then sum stages feed the store stream
                load_x1(0)
                load_x1(1)
                load_x0_half(0, 0)
                load_x0_half(1, 0)
                load_x0_half(0, 1)
                load_x0_half(1, 1)
                ch0 = seg_counts(am[0], "h0")
                transpose_a(0)
                cv0 = seg_counts(aT[0], "v0")
                # the chain that opens the store stream gets top scheduling
                # priority so ready-but-noncritical work can't delay it
                with tc.high_priority():
                    transpose_x(0)
                    yv[0], fin_v0 = seg_sums_split(xT[0], aT[0], cv0, "v0")
                    yh[0], fin_h0 = seg_sums_split(xdv(0), am[0], ch0, "h0")
                    combine_store(0, 0, first=True)  # store stream opens
                fin_v0()
                fin_h0()
                ch1 = seg_counts(am[1], "h1")
                yh[1] = seg_sums(xdv(1), am[1], ch1, "h1")
                combine_store(1, 0)
                transpose_a(1)
                cv1 = seg_counts(aT[1], "v1")
                transpose_x(1)
                yv[1] = seg_sums(xT[1], aT[1], cv1, "v1")
                combine_store(0, 1)
                combine_store(1, 1)

            if loop_n > 1:
                with tc.For_i(0, loop_n, 1):
                    body()
            else:
                body()

    nc.compile()
    return nc


def _get_nc():
    if "nc" not in _CACHE:
        _CACHE["nc"] = build_program()
    return _CACHE["nc"]


def kernel(x0: np.ndarray, x1: np.ndarray) -> np.ndarray:
    from concourse.bass_utils import run_bass_kernel_spmd

    nc = _get_nc()
    n = x0.shape[0]
    in_maps = [
        {"x0": np.ascontiguousarray(x0[i, 0]),
         "x1": np.ascontiguousarray(x1[i, 0])}
        for i in range(n)
    ]
    res = run_bass_kernel_spmd(nc, in_maps, list(range(N_CORES)))
    out = np.stack([res.results[i]["y"] for i in range(n)])
    return out.reshape(n, 1, H, W).astype(np.float32)



# revision 2
# speedup vs baseline: 1.0087x; 1.0087x over previous
"""Trainium2 Bass kernel for nn_BAFM_BRB_65249143161427 (segment_reduce).

v2: front-compressed schedule. The 16 MB output store stream is the
roofline (~47 us); total time = time-to-first-store + stream + tail.
Changes vs v1:
  - x1 loaded in column halves; the 2 tiles' h1/last x0 half deferred
    until after the first store block is issued (3.0 MB instead of 4 MB
    gates the store stream).
  - critical chain to the first store emitted in dependency order at
    naturally increasing priority; all later work priority-bumped to
    1000+ so the scheduler cannot interleave it into the chain.
  - v0/h0 scan tails (mul/mul/add) offloaded to Pool so DVE is free for
    the next scans; first store block's expansion split into column
    halves so the stream opens ~0.5 us earlier.
"""
import sys

sys.path.insert(0, "/opt/trn_rl_repo")

import numpy as np

H = W = 2048
S = 8
HD, WD = H // S, W // S      # 256 x 256 downsampled grid
P = 128                      # SBUF partitions
NT = HD // P                 # 2 row-tiles of the downsampled grid
N_CORES = 8
HW_ = W // 2                 # raw column half
HD_ = WD // 2                # downsampled column half

_CACHE = {}


def _revap(ap2d):
    """Reverse the last (free) dim of a 2D access pattern."""
    from concourse.ap import AP

    pairs = [list(p) for p in ap2d.ap]
    step, n = pairs[-1]
    return AP(ap2d.tensor, ap2d.offset + (n - 1) * step, pairs[:-1] + [[-step, n]])


def _bcast8(ap2d):
    """Append a step-0 count-8 inner dim (each element replicated 8x)."""
    from concourse.ap import AP

    pairs = [list(p) for p in ap2d.ap]
    return AP(ap2d.tensor, ap2d.offset, pairs + [[0, 8]])


def build_program(loop_n: int = 1):
    import concourse.bacc as bacc
    import concourse.tile as tile
    from concourse import mybir
    from concourse.masks import make_identity
    from contextlib import ExitStack

    f32 = mybir.dt.float32
    i32 = mybir.dt.int32
    MUL = mybir.AluOpType.mult
    ADD = mybir.AluOpType.add
    NE = mybir.AluOpType.not_equal

    # Bacc (not raw Bass): its compile() splits multi-wait sync commands,
    # which TRN2 engines (1 wait/instruction) require.
    nc = bacc.Bacc("TRN2")
    x0 = nc.declare_dram_parameter("x0", [H, W], f32, isOutput=False)
    x1 = nc.declare_dram_parameter("x1", [H, W], i32, isOutput=False)
    y = nc.declare_dram_parameter("y", [H, W], f32, isOutput=True)

    # DRAM viewed with rows grouped by 8: [256, 8, 2048]
    x0g = x0[:].rearrange("(r e) w -> r e w", e=8)
    x1g = x1[:].rearrange("(r e) w -> r e w", e=8)
    yg = y[:].rearrange("(r e) w -> r e w", e=8)

    with tile.TileContext(nc) as tc:
        with ExitStack() as ctx:
            const_pool = ctx.enter_context(tc.tile_pool(name="const", bufs=1))
            io_pool = ctx.enter_context(tc.tile_pool(name="io", bufs=1))
            work = ctx.enter_context(tc.tile_pool(name="work", bufs=1))
            psum = ctx.enter_context(
                tc.tile_pool(name="psum", bufs=2, space="PSUM")
            )
            psum_x = ctx.enter_context(
                tc.tile_pool(name="psum_x", bufs=2, space="PSUM")
            )

            ident_g = const_pool.tile([P, P], f32)
            ones = const_pool.tile([P, WD], f32)
            ident = const_pool.tile([P, P], f32)

            def body():
                # ---- tiles ----
                am = [work.tile([P, WD], f32, tag=f"am{t}", name=f"am{t}") for t in range(NT)]
                xT = [psum_x.tile([P, HD], f32, tag=f"xT{c}", name=f"xT{c}") for c in range(NT)]
                aT = [work.tile([P, HD], f32, tag=f"aT{c}", name=f"aT{c}") for c in range(NT)]
                yh = [None] * NT    # horizontal-pass results
                yv = [None] * NT    # vertical-pass results (transposed)
                ye = [io_pool.tile([P, W], f32, tag=f"ye{t}", name=f"ye{t}") for t in range(NT)]
                x1r = [io_pool.tile([P, W], i32, tag=f"x1r{t}", name=f"x1r{t}")
                       for t in range(NT)]
                x0r = [io_pool.tile([P, W], f32, tag=f"x0r{t}", name=f"x0r{t}")
                       for t in range(NT)]

                def load_half(dst, src, t, h):
                    nc.sync.dma_start(
                        out=dst[t][:, h * HW_:(h + 1) * HW_],
                        in_=src[t * P:(t + 1) * P, 0, h * HW_:(h + 1) * HW_],
                    )

                def mask_half(t, h):
                    # (DVE: Pool rejects TensorScalarPtr in codegen)
                    nc.vector.tensor_scalar(
                        out=am[t][:, h * HD_:(h + 1) * HD_],
                        in0=x1r[t][:, h * HW_:(h + 1) * HW_].rearrange(
                            "p (a b) -> p a b", b=8)[:, :, 0],
                        scalar1=0, scalar2=None, op0=NE,
                    )

                def xdv(t, lo=0, hi=WD):
                    """Stride-8 view of x0r[t] covering downsampled cols
                    [lo, hi)."""
                    return x0r[t][:, lo * 8:hi * 8].rearrange(
                        "p (a b) -> p a b", b=8)[:, :, 0]

                def seg_counts(at, pf, split=False):
                    """Mask-only stage: count reciprocals + invalid mask q.
                    split=True emits only the halves needed by block column 0
                    (ilr[0:P], irl mirror, q[0:P]) and returns a closure that
                    emits the complements (call before using block column 1).
                    """
                    c_lr = work.tile([P, WD], f32, tag=f"clr{pf}")
                    nb = work.tile([P, WD], f32, tag=f"nb{pf}")
                    c_rl = work.tile([P, WD], f32, tag=f"crl{pf}")  # reversed
                    na = work.tile([P, WD], f32, tag=f"na{pf}")     # reversed
                    a_r = _revap(at[:])
                    ttscan = nc.vector.tensor_tensor_scan
                    ttscan(c_lr[:], at[:], ones[:], 0.0, MUL, ADD)
                    ttscan(c_rl[:], a_r, ones[:], 0.0, MUL, ADD)
                    ttscan(nb[:], at[:], at[:], 1.0, MUL, MUL)
                    ttscan(na[:], a_r, a_r, 1.0, MUL, MUL)
                    ilr = work.tile([P, WD], f32, tag=f"ilr{pf}")
                    irl = work.tile([P, WD], f32, tag=f"irl{pf}")
                    q = work.tile([P, WD], i32, tag=f"q{pf}")
                    # fast variant: counts are small integers, and one DVE op
                    # per reciprocal instead of two keeps DVE clear for the
                    # critical scans (tolerance is 2e-2; approx_fast is ~1e-4)
                    recip = nc.vector.reciprocal_approx_fast

                    def emit(lo, hi):
                        rlo, rhi = WD - hi, WD - lo
                        recip(ilr[:, lo:hi], c_lr[:, lo:hi])
                        recip(irl[:, rlo:rhi], c_rl[:, rlo:rhi])
                        # q = nb + reverse(na): nonzero -> no enclosing bound
                        nc.vector.tensor_tensor(
                            q[:, lo:hi], nb[:, lo:hi],
                            _revap(na[:, rlo:rhi]), ADD)

                    if not split:
                        emit(0, WD)
                        return ilr, irl, q
                    emit(0, P)
                    return (ilr, irl, q), (lambda: emit(P, WD))

                def seg_scans(xt, at, pf, rl_first=False, skip_lr=False):
                    """Value-stage scans only (DVE). rl_first puts the
                    reversed scan ahead (its consumer chain is longer);
                    skip_lr leaves the forward scan to the caller (partial
                    prefix scans are self-contained)."""
                    s_lr = work.tile([P, WD], f32, tag=f"slr{pf}")
                    s_rl = work.tile([P, WD], f32, tag=f"srl{pf}")  # reversed
                    ttscan = nc.vector.tensor_tensor_scan

                    def lr():
                        ttscan(s_lr[:], at[:], xt[:], 0.0, MUL, ADD)

                    def rl():
                        ttscan(s_rl[:], _revap(at[:]), _revap(xt[:]),
                               0.0, MUL, ADD)

                    if rl_first:
                        rl()
                        if not skip_lr:
                            lr()
                    else:
                        if not skip_lr:
                            lr()
                        rl()
                    two_x = work.tile([P, WD], f32, tag=f"tx{pf}")
                    nc.scalar.mul(two_x[:], xt[:], 2.0)
                    m = work.tile([P, WD], f32, tag=f"m{pf}")
                    return s_lr, s_rl, two_x, m

                def seg_finish_cols(sc, cnts, lo, hi, eng,
                                    srl_first=False, slr_done=False):
                    """Finish downsampled cols [lo, hi): means + fallback.
                    eng does the elementwise math; copy_predicated is DVE.
                    srl_first: multiply the reversed sums first (their scan
                    completes first). slr_done: caller already scaled
                    s_lr[:, lo:hi] in place."""
                    s_lr, s_rl, two_x, m = sc
                    ilr, irl, q = cnts
                    rlo, rhi = WD - hi, WD - lo  # mirrored slice (rev space)

                    def mul_lr():
                        eng.tensor_tensor(
                            s_lr[:, lo:hi], s_lr[:, lo:hi], ilr[:, lo:hi], MUL)

                    def mul_rl():
                        eng.tensor_tensor(
                            s_rl[:, rlo:rhi], s_rl[:, rlo:rhi],
                            irl[:, rlo:rhi], MUL)

                    if srl_first:
                        mul_rl()
                        if not slr_done:
                            mul_lr()
                    else:
                        if not slr_done:
                            mul_lr()
                        mul_rl()
                    eng.tensor_tensor(
                        m[:, lo:hi], s_lr[:, lo:hi],
                        _revap(s_rl[:, rlo:rhi]), ADD)
                    nc.vector.copy_predicated(
                        m[:, lo:hi], q[:, lo:hi], two_x[:, lo:hi])
                    return m

                def seg_finish(sc, cnts, pf, h, eng, srl_first=False):
                    hd_ = WD // 2
                    return seg_finish_cols(sc, cnts, h * hd_, (h + 1) * hd_,
                                           eng, srl_first=srl_first)

                def seg_sums(xt, at, cnts, pf, tail_eng=None):
                    """Value stage: segment sums -> means -> m with fallback."""
                    te = tail_eng or nc.gpsimd
                    sc = seg_scans(xt, at, pf)
                    seg_finish(sc, cnts, pf, 0, te)
                    return seg_finish(sc, cnts, pf, 1, te)

                def transpose_a(c, eng=None):
                    """Transposed mask -> aT[c] (SBUF, bounced via PSUM:
                    scan data0 and data1 cannot both live in PSUM)."""
                    for t in range(NT):
                        pb = psum.tile([P, P], f32, tag="ptr")
                        nc.tensor.transpose(
                            pb[:], am[t][:, c * P:(c + 1) * P], ident[:]
                        )
                        if eng is None:
                            nc.scalar.copy(aT[c][:, t * P:(t + 1) * P], pb[:])
                        else:
                            eng.tensor_copy(aT[c][:, t * P:(t + 1) * P], pb[:])

                def transpose_x(c):
                    """Transpose x straight into the PSUM tile the vertical
                    sum-scans read."""
                    for t in range(NT):
                        nc.tensor.transpose(
                            xT[c][:, t * P:(t + 1) * P],
                            xdv(t, c * P, (c + 1) * P), ident[:],
                        )

                def store_block(t, c, lo, hi, issuers):
                    """DMA out the 8 row-replicas of ye[t] downsampled cols
                    [lo, hi) of block column c."""
                    glo, ghi = lo * 8, hi * 8
                    for k in range(8):
                        issuer = issuers[k % len(issuers)]
                        issuer.dma_start(
                            out=yg[t * P:(t + 1) * P, k, glo:ghi],
                            in_=ye[t][:, glo:ghi],
                        )

                def combine_store(t, c):
                    """y block (t,c) = yh[t][:,cP:] + yv[c][:,tP:]^T;
                    expand 8x8, store 8 row-replicas."""
                    pb = psum.tile([P, P], f32, tag="ptb")
                    nc.tensor.transpose(
                        pb[:], yv[c][:, t * P:(t + 1) * P], ident[:]
                    )
                    ysum = work.tile([P, P], f32, tag=f"ys{t}{c}")
                    nc.vector.tensor_tensor(
                        ysum[:], yh[t][:, c * P:(c + 1) * P], pb[:], ADD
                    )
                    ye_view = ye[t][:, c * P * 8:(c + 1) * P * 8].rearrange(
                        "p (a b) -> p a b", b=8)
                    nc.scalar.copy(ye_view, _bcast8(ysum[:]))
                    store_block(t, c, c * P, (c + 1) * P, (nc.sync, nc.scalar))

                # ---- loads: everything the first store block needs ----
                load_half(x1r, x1g, 0, 0)
                load_half(x1r, x1g, 0, 1)
                load_half(x1r, x1g, 1, 0)
                load_half(x0r, x0g, 0, 0)
                load_half(x0r, x0g, 1, 0)
                load_half(x0r, x0g, 0, 1)

                # ---- critical chain to the first store, in dep order ----
                mask_half(0, 0)
                mask_half(0, 1)
                # h0 count scans only need am[0] (fills DVE while
                # x1(1,h0) is in flight)
                ch0 = seg_counts(am[0], "h0")
                mask_half(1, 0)
                transpose_a(0)
                # early partial forward h-scan: cols [0:128) of a forward
                # segment scan are self-contained (left boundaries cannot
                # reach past col 0), so this is gated only by x0(0,h0);
                # scale the two first-block quarters in place right away
                sc_h0 = (
                    work.tile([P, WD], f32, tag="slrh0", name="slrh0"),
                    work.tile([P, WD], f32, tag="srlh0", name="srlh0"),
                    work.tile([P, WD], f32, tag="txh0", name="txh0"),
                    work.tile([P, WD], f32, tag="mh0", name="mh0"),
                )
                s_lr_h0 = sc_h0[0]
                nc.vector.tensor_tensor_scan(
                    s_lr_h0[:, 0:P], am[0][:, 0:P], xdv(0, 0, P),
                    0.0, MUL, ADD)
                for qi in (0, 1):
                    lo, hi = qi * 64, (qi + 1) * 64
                    nc.vector.tensor_tensor(
                        s_lr_h0[:, lo:hi], s_lr_h0[:, lo:hi],
                        ch0[0][:, lo:hi], MUL)
                # early partial two_x for the first block's quarters
                nc.scalar.mul(sc_h0[2][:, 0:P], xdv(0, 0, P), 2.0)
                cv0 = seg_counts(aT[0], "v0")
                transpose_x(0)
                # deferred loads stream through the DMA-idle latency window
                # between the last critical load and the first store
                load_half(x1r, x1g, 1, 1)
                load_half(x0r, x0g, 1, 1)
                # v-scans with the reversed scan first (longer consumer
                # chain), then the gating reversed h-scan; v0 half-0 finish
                # on Pool
                sc_v0 = seg_scans(xT[0], aT[0], "v0", rl_first=True)
                yv[0] = seg_finish(sc_v0, cv0, "v0", 0, nc.gpsimd,
                                   srl_first=True)
                m_v0 = yv[0]
                # gating reversed h-scan (needs the full x0 row)
                nc.vector.tensor_tensor_scan(
                    sc_h0[1][:], _revap(am[0][:]), _revap(xdv(0)),
                    0.0, MUL, ADD)
                pb00 = psum.tile([P, P], f32, tag="ptb")
                nc.tensor.transpose(pb00[:], m_v0[:, 0:P], ident[:])
                # h-pass finish in 64-col quarters: mul/add on Pool,
                # predication + combine + expansion on DVE; the stream opens
                # on the first quarter's 8 stores
                for qi in (0, 1):
                    lo, hi = qi * 64, (qi + 1) * 64
                    yh[0] = seg_finish_cols(sc_h0, ch0, lo, hi, nc.vector,
                                            slr_done=True)
                    ysum = work.tile([P, 64], f32, tag=f"ys00{qi}",
                                     name=f"ys00{qi}")
                    nc.vector.tensor_tensor(
                        ysum[:], yh[0][:, lo:hi], pb00[:, lo:hi], ADD)
                    ye_view = ye[0][:, lo * 8:hi * 8].rearrange(
                        "p (a b) -> p a b", b=8)
                    if qi == 0:
                        nc.vector.tensor_copy(ye_view, _bcast8(ysum[:]))
                    else:
                        nc.scalar.copy(ye_view, _bcast8(ysum[:]))
                    store_block(0, 0, lo, hi, (nc.sync, nc.scalar))
                # store stream is open

                # ---- rest at low priority + a scheduler time fence so it
                # cannot be slotted into the critical chain's stall windows
                tc.cur_priority = max(tc.cur_priority, 1000)
                if True:
                    # block (1,0) first: earliest stream deadline
                    mask_half(1, 1)
                    ch1 = seg_counts(am[1], "h1")
                    yh[1] = seg_sums(xdv(1), am[1], ch1, "h1")
                    seg_finish(sc_v0, cv0, "v0", 1, nc.vector)  # feeds (1,0)
                    combine_store(1, 0)
                    # full forward h0 scan (recomputes cols [0:128)
                    # identically, in place) + the remaining h-pass half
                    nc.vector.tensor_tensor_scan(
                        s_lr_h0[:], am[0][:], xdv(0), 0.0, MUL, ADD)
                    nc.scalar.mul(sc_h0[2][:, P:WD], xdv(0, P, WD), 2.0)
                    seg_finish(sc_h0, ch0, "h0", 1, nc.vector)  # feeds (0,1)
                    transpose_a(1)
                    cv1 = seg_counts(aT[1], "v1")
                    transpose_x(1)
                    yv[1] = seg_sums(xT[1], aT[1], cv1, "v1")
                    combine_store(0, 1)
                    combine_store(1, 1)

            def consts():
                make_identity(nc, ident_g[:])
                nc.vector.tensor_copy(ident[:], ident_g[:])
                nc.gpsimd.memset(ones[:], 1.0)

            if loop_n > 1:
                consts()
                with tc.For_i(0, loop_n, 1):
                    body()
            else:
                consts()
                body()

    nc.compile()
    return nc


def _get_nc():
    if "nc" not in _CACHE:
        _CACHE["nc"] = build_program()
    return _CACHE["nc"]


def kernel(x0: np.ndarray, x1: np.ndarray) -> np.ndarray:
    from concourse.bass_utils import run_bass_kernel_spmd

    nc = _get_nc()
    n = x0.shape[0]
    in_maps = [
        {"x0": np.ascontiguousarray(x0[i, 0]),
         "x1": np.ascontiguousarray(x1[i, 0])}
        for i in range(n)
    ]
    res = run_bass_kernel_spmd(nc, in_maps, list(range(N_CORES)))
    out = np.stack([res.results[i]["y"] for i in range(n)])
    return out.reshape(n, 1, H, W).astype(np.float32)
